# revision 33
# baseline (speedup 1.0000x reference)
"""Cross-attention Bass kernel for 8 trn2 NeuronCores.

Sharding: core d handles batch b = d//4 and query rows [(d%4)*1024, (d%4+1)*1024)
of that batch, computing all 8 heads (no collectives). The context is compacted
on the host using the mask (masked rows dropped, zero-padded to a multiple of
128), which preserves softmax semantics exactly while nearly halving the
attention work.

Device dataflow (k-blocked, engine-balanced, software-pipelined):
  - Q^T = Wq^T x^T (bf16, softmax scale folded into the drain), K^T = Wk^T ctx^T
    (bf16), V natural = ctx^T-contracted with Wv (bf16) with a per-head "ones"
    column carrying the valid mask.
  - Scores per 2-ktile group, split by head into two 2-bank PSUM tiles so the
    exp of head A overlaps the score matmuls of head B (effective double
    buffering inside the 8-bank budget); one exp per head-group on ScalarE
    -> P^T bf16.
  - PV in NATURAL orientation: O[q, 65] += P^T-chunk.T @ [V|valid] with bf16
    operands (free dim 65 -> half the PE rows of the O^T orientation). Each
    PSUM bank holds 4 q-chunk chains as ONE accumulation group (start on the
    bank's first matmul, stop on its last). Chains span a 2-4 ktile
    super-block, then drain-add into an SBUF O accumulator on VectorE. The
    softmax denominator rides along as column 64, so normalization is a
    per-partition scalar multiply - no DRAM broadcast round-trips.
  - PV and drain-adds are emitted one group late (software pipeline), hiding
    the exp latency; K/V/ctx^T projection work is split into small units and
    deadline-paced into the attention group loop so the Tensor engine stays
    busy while ScalarE exps.
  - Epilogue: normalize on VectorE+ScalarE into packed tiles, PE-transpose,
    output projection, bias on VectorE, store via Pool-engine DMA.
"""
import numpy as np

B, N, M = 2, 4096, 4096
QUERY_DIM, CONTEXT_DIM = 512, 768
H, D = 8, 64
INNER = H * D  # 512
NCORES = 8
N_DEV = (B * N) // NCORES  # 1024 query rows per core
M_PAD_MIN = 256

_compiled = {}


def _build(m_pad):
    from collections import deque

    from concourse import bacc
    import concourse.bass as bass
    import concourse.mybir as mybir
    import concourse.tile as tile
    from concourse.masks import make_identity

    F32 = mybir.dt.float32
    F32R = mybir.dt.float32r
    BF16 = mybir.dt.bfloat16
    AF = mybir.ActivationFunctionType

    KT = m_pad // 128
    SCALE = float(D) ** -0.5
    CQ = QUERY_DIM // 128  # 4
    CC = CONTEXT_DIM // 128  # 6
    CI = INNER // 128  # 4
    NQT = N_DEV // 128  # 8
    QB = 512
    NQB = N_DEV // QB  # 2

    # super-blocks (PV accumulation chain extents): small first block so
    # attention starts early, then 4-ktile blocks
    sbs = []
    s = 0
    while s < KT:
        n = min(2 if s == 0 else 4, KT - s)
        sbs.append((s, n))
        s += n
    # projection j-blocks: pairs of ktiles; fold an odd trailing ktile into a
    # final 3-wide block so every K-projection matmul keeps free dim >= 256
    if KT % 2 == 1 and KT >= 3:
        jbs = [(b, 2) for b in range(0, KT - 3, 2)] + [(KT - 3, 3)]
    else:
        jbs = [(b, min(2, KT - b)) for b in range(0, KT, 2)]
    JW = 384 if (KT % 2 == 1 and KT >= 3) else 256  # widest j-block

    nc = bacc.Bacc()
    xs_d = nc.declare_dram_parameter("xs", [N_DEV, QUERY_DIM], BF16, isOutput=False)
    ctx_d = nc.declare_dram_parameter("ctx", [m_pad, CONTEXT_DIM], BF16, isOutput=False)
    val_d = nc.declare_dram_parameter("valid", [m_pad], F32, isOutput=False)
    wq_d = nc.declare_dram_parameter("Wq", [QUERY_DIM, INNER], BF16, isOutput=False)
    wk_d = nc.declare_dram_parameter("Wk", [CONTEXT_DIM, INNER], BF16, isOutput=False)
    wv_d = nc.declare_dram_parameter("Wv", [CONTEXT_DIM, INNER], BF16, isOutput=False)
    wo_d = nc.declare_dram_parameter("Wo", [INNER, QUERY_DIM], BF16, isOutput=False)
    bo_d = nc.declare_dram_parameter("bo", [QUERY_DIM], F32, isOutput=False)
    out_d = nc.declare_dram_parameter("out", [N_DEV, QUERY_DIM], F32, isOutput=True)

    with tile.TileContext(nc) as tc:
        with (
            tc.tile_pool(name="big", bufs=1) as big,
            tc.tile_pool(name="strm", bufs=3) as strm,
            tc.tile_pool(name="ld", bufs=4) as ld,
            tc.tile_pool(name="ptp", bufs=8) as ptp,
            tc.tile_pool(name="outp", bufs=2) as outp,
            tc.tile_pool(name="nrm", bufs=4) as nrm,
            tc.tile_pool(name="ps_scA", bufs=1, space="PSUM") as ps_scA,
            tc.tile_pool(name="ps_scB", bufs=1, space="PSUM") as ps_scB,
            tc.tile_pool(name="ps_pv", bufs=2, space="PSUM") as ps_pv,
            tc.tile_pool(name="ps_pj", bufs=2, space="PSUM") as ps_pj,
        ):
            # ---- persistent SBUF tiles ----
            qT = big.tile([128, CI, N_DEV], BF16, tag="qT", name="qT")
            kT = big.tile([128, CI, m_pad], BF16, tag="kT", name="kT")
            v2 = [
                big.tile([128, H * 65], BF16, tag=f"v2_{t}", name=f"v2_{t}")
                for t in range(KT)
            ]
            # O accumulator: [q-tile, head, 64 dims + denominator]
            O = big.tile([128, NQT, H, 65], F32R, tag="O", name="O")
            rcp = big.tile([128, NQT, H], F32, tag="rcp", name="rcp")
            wo = big.tile([128, CI, QUERY_DIM], BF16, tag="wo", name="wo")
            bo_bc = big.tile([128, QUERY_DIM], F32, tag="bo", name="bo")
            valid = big.tile([128, KT], F32, tag="valid", name="valid")
            oT = [
                big.tile([128, CI, QB], BF16, tag=f"oT{qb}", name=f"oT{qb}")
                for qb in range(NQB)
            ]
            wk = big.tile([128, CC, INNER], BF16, tag="wk", name="wk")
            wv = big.tile([128, CC, INNER], BF16, tag="wv", name="wv")
            identf = big.tile([128, 128], F32, tag="identf", name="identf")
            identb = big.tile([128, 128], BF16, tag="identb", name="identb")

            # ---- prologue: x^T and Q^T ----
            xs = big.tile([128, NQT, QUERY_DIM], BF16, tag="xs", name="xs")
            xT = big.tile([128, CQ, N_DEV], BF16, tag="xT", name="xT")
            wq = big.tile([128, CQ, INNER], BF16, tag="wq", name="wq")
            if True:
                xs_r = xs_d[:].rearrange("(t p) f -> p t f", p=128)
                nc.sync.dma_start(out=xs[:, 0:4, :], in_=xs_r[:, 0:4, :])
                nc.sync.dma_start(
                    out=wq[:], in_=wq_d[:].rearrange("(o p) f -> p o f", p=128)
                )
                nc.sync.dma_start(
                    out=wk[:], in_=wk_d[:].rearrange("(o p) f -> p o f", p=128)
                )
                nc.sync.dma_start(out=xs[:, 4:8, :], in_=xs_r[:, 4:8, :])
                nc.sync.dma_start(
                    out=wv[:], in_=wv_d[:].rearrange("(o p) f -> p o f", p=128)
                )
                nc.sync.dma_start(
                    out=valid[:], in_=val_d[:].rearrange("(t p) -> p t", p=128)
                )
                nc.sync.dma_start(
                    out=bo_bc[:],
                    in_=bass.AP(tensor=bo_d, offset=0, ap=[[0, 128], [1, QUERY_DIM]]),
                )
                make_identity(nc, identf[:])
                nc.vector.tensor_copy(identb[:], identf[:])

                # x^T then Q^T for the first q-half only; the second half is
                # deferred into filler units (qb1 consumes it several slots in)
                for qf in range(1):
                    for nt in range(qf * 4, qf * 4 + 4):
                        if nt % 2 == 0:
                            dst = ps_pj.tile([128, 512], BF16, tag="pj", name="pjx")
                        else:
                            dst = ps_pv.tile([128, 512], BF16, tag="pv", name="pvx")
                        for c in range(CQ):
                            nc.tensor.transpose(
                                dst[:, c * 128 : (c + 1) * 128],
                                xs[:, nt, c * 128 : (c + 1) * 128],
                                identb[:],
                            )
                        nc.vector.tensor_copy(
                            xT[:, :, nt * 128 : (nt + 1) * 128],
                            dst[:, 0 : CQ * 128].rearrange("p (c n) -> p c n", n=128),
                        )
                    for dc in range(CI):
                        psq = ps_pv.tile([128, 512], F32, tag="pv", name="psq")
                        for c in range(CQ):
                            nc.tensor.matmul(
                                psq[:],
                                wq[:, c, dc * 128 : (dc + 1) * 128],
                                xT[:, c, qf * QB : (qf + 1) * QB],
                                start=(c == 0),
                                stop=(c == CQ - 1),
                            )
                        nc.scalar.activation(
                            qT[:, dc, qf * QB : (qf + 1) * QB],
                            psq[:],
                            AF.Copy,
                            scale=SCALE,
                        )

            # ---- projection units (ctx^T, K^T, V), deadline-paced below ----
            ctxT_tiles = {}

            def mk_T(jbi, k):
                def t_unit():
                    b, nkt = jbs[jbi]
                    if k == 0:
                        ctxT_tiles[jbi] = strm.tile(
                            [128, CC, JW], BF16, tag="ctxT", name="ctxT"
                        )
                    ct = ctxT_tiles[jbi]
                    t = b + k
                    raw = ld.tile([128, CONTEXT_DIM], BF16, tag="ld", name="ld")
                    nc.sync.dma_start(
                        out=raw[:], in_=ctx_d[t * 128 : (t + 1) * 128, :]
                    )
                    for c0, cn in ((0, 4), (4, 2)):
                        pj = ps_pj.tile([128, 512], BF16, tag="pj", name="pjt")
                        for cl in range(cn):
                            nc.tensor.transpose(
                                pj[:, cl * 128 : (cl + 1) * 128],
                                raw[:, (c0 + cl) * 128 : (c0 + cl + 1) * 128],
                                identb[:],
                            )
                        nc.vector.tensor_copy(
                            ct[:, c0 : c0 + cn, k * 128 : (k + 1) * 128],
                            pj[:, 0 : cn * 128].rearrange("p (c n) -> p c n", n=128),
                        )

                return t_unit

            kpj = {}

            def mk_K(jbi, dc, half):
                # K^T for j-block jbi, INNER chunk dc, contraction half
                def k_unit():
                    b, nkt = jbs[jbi]
                    w = nkt * 128
                    ct = ctxT_tiles[jbi]
                    if half == 0:
                        kpj[(jbi, dc)] = ps_pj.tile(
                            [128, 512], F32, tag="pj", name="pjk"
                        )
                    pj = kpj[(jbi, dc)]
                    for c in range(half * 3, half * 3 + 3):
                        nc.tensor.matmul(
                            pj[:, 0:w],
                            wk[:, c, dc * 128 : (dc + 1) * 128],
                            ct[:, c, 0:w],
                            start=(c == 0),
                            stop=(c == CC - 1),
                        )
                    if half == 1:
                        nc.vector.tensor_copy(
                            kT[:, dc, b * 128 : b * 128 + w], pj[:, 0:w]
                        )

                return k_unit

            vpj = {}

            def mk_V(jbi, k, half):
                def v_unit():
                    b, nkt = jbs[jbi]
                    ct = ctxT_tiles[jbi]
                    t = b + k
                    if half == 0:
                        vpj[t] = ps_pj.tile([128, 512], F32, tag="pj", name="pjv")
                    sl = vpj[t][:, 0:512]
                    for c in range(half * 3, half * 3 + 3):
                        nc.tensor.matmul(
                            sl,
                            ct[:, c, k * 128 : (k + 1) * 128],
                            wv[:, c, :],
                            start=(c == 0),
                            stop=(c == CC - 1),
                        )
                    if half == 1:
                        v2h = v2[t][:].rearrange("p (h c) -> p h c", c=65)
                        nc.vector.tensor_copy(
                            v2h[:, :, 0:64],
                            sl.rearrange("p (h d) -> p h d", d=64),
                        )
                        nc.gpsimd.tensor_copy(
                            v2h[:, :, 64:65],
                            valid[:, t : t + 1].to_broadcast([128, H, 1]),
                        )

                return v_unit

            # (unit_fn, kt_covered_after) - coverage advances when the last
            # unit of a j-block has been emitted
            units = deque()
            for jbi, (b, nkt) in enumerate(jbs):
                seq = (
                    [mk_T(jbi, k) for k in range(nkt)]
                    + [mk_K(jbi, dc, h) for dc in range(CI) for h in range(2)]
                    + [mk_V(jbi, k, h) for k in range(nkt) for h in range(2)]
                )
                for i, fn in enumerate(seq):
                    units.append((fn, b + nkt if i == len(seq) - 1 else 0))

            def mk_xq(step):
                def xq_unit():
                    if step == 0:
                        for nt in range(4, 8):
                            dst = (ps_pj if nt % 2 == 0 else ps_pv).tile(
                                [128, 512], BF16,
                                tag="pj" if nt % 2 == 0 else "pv",
                                name="pvx2",
                            )
                            for c in range(CQ):
                                nc.tensor.transpose(
                                    dst[:, c * 128 : (c + 1) * 128],
                                    xs[:, nt, c * 128 : (c + 1) * 128],
                                    identb[:],
                                )
                            nc.vector.tensor_copy(
                                xT[:, :, nt * 128 : (nt + 1) * 128],
                                dst[:, 0 : CQ * 128].rearrange(
                                    "p (c n) -> p c n", n=128
                                ),
                            )
                    else:
                        dc = step - 1
                        psq = ps_pv.tile([128, 512], F32, tag="pv", name="psq2")
                        for c in range(CQ):
                            nc.tensor.matmul(
                                psq[:],
                                wq[:, c, dc * 128 : (dc + 1) * 128],
                                xT[:, c, QB : 2 * QB],
                                start=(c == 0),
                                stop=(c == CQ - 1),
                            )
                        nc.scalar.activation(
                            qT[:, dc, QB : 2 * QB], psq[:], AF.Copy, scale=SCALE
                        )

                return xq_unit

            for step in range(4, -1, -1):
                units.appendleft((mk_xq(step), 0))

            coverage = [0]

            def pop_unit():
                fn, cov = units.popleft()
                fn()
                if cov:
                    coverage[0] = cov

            # prologue: emit units covering super-block 0
            while units and coverage[0] < sbs[0][1]:
                pop_unit()

            nc.sync.dma_start(
                out=wo[:], in_=wo_d[:].rearrange("(o p) f -> p o f", p=128)
            )

            # ---- epilogue units ----
            onrm = {}
            epi = deque()

            def mk_norm(qt):
                def n_unit():
                    nc.vector.reciprocal(
                        rcp[:, qt : qt + 1, :], O[:, qt : qt + 1, :, 64:65]
                    )
                    st = nrm.tile([128, INNER], BF16, tag="onrm", name="onrm")
                    onrm[qt] = st
                    for h in range(H):
                        nc.vector.tensor_scalar_mul(
                            st[:, h * 64 : (h + 1) * 64],
                            O[:, qt : qt + 1, h : h + 1, 0:64],
                            rcp[:, qt : qt + 1, h : h + 1],
                        )

                return n_unit

            def mk_tr(qt):
                def tr_unit():
                    qb, qtl = qt // (NQT // NQB), qt % (NQT // NQB)
                    pj = ps_pj.tile([128, 512], BF16, tag="pj", name="pjtr")
                    for c in range(CI):
                        nc.tensor.transpose(
                            pj[:, c * 128 : (c + 1) * 128],
                            onrm[qt][:, c * 128 : (c + 1) * 128],
                            identb[:],
                        )
                    nc.vector.tensor_copy(
                        oT[qb][:, :, qtl * 128 : (qtl + 1) * 128],
                        pj[:, 0 : CI * 128].rearrange("p (c n) -> p c n", n=128),
                    )

                return tr_unit

            def mk_op(qt):
                def op_unit():
                    qb, qtl = qt // (NQT // NQB), qt % (NQT // NQB)
                    pj = ps_pj.tile([128, 512], F32, tag="pj", name="pjop")
                    for c in range(CI):
                        nc.tensor.matmul(
                            pj[:, 0:512],
                            oT[qb][:, c, qtl * 128 : (qtl + 1) * 128],
                            wo[:, c, :],
                            start=(c == 0),
                            stop=(c == CI - 1),
                        )
                    ot = outp.tile([128, QUERY_DIM], F32, tag="ot", name="ot")
                    nc.vector.tensor_add(ot[:], pj[:, 0:512], bo_bc[:])
                    nc.gpsimd.dma_start(
                        out=out_d[qt * 128 : (qt + 1) * 128, :], in_=ot[:]
                    )

                return op_unit

            # ---- main attention loop (software-pipelined across groups,
            # iterations, and super-blocks) ----
            sched = []
            for sbi, (s0, sn) in enumerate(sbs):
                groups = [(t0, min(2, s0 + sn - t0)) for t0 in range(s0, s0 + sn, 2)]
                for qb in range(NQB):
                    for hp in range(H // 2):
                        sched.append((sbi, s0, sn, qb, hp, groups))

            def emit_pv(t0, tn, pts, pvs, heads, s0, send):
                for hi, (h, pv) in enumerate(zip(heads, pvs)):
                    for j in range(tn):
                        t = t0 + j
                        for qc in range(4):
                            # one accumulation group per PSUM bank: start/stop
                            # only on the bank's first/last matmul of the sb
                            nc.tensor.matmul(
                                pv[:, qc * 128 : qc * 128 + 65],
                                pts[hi][:, j, qc * 128 : (qc + 1) * 128],
                                v2[t][:, h * 65 : h * 65 + 65],
                                start=(t == s0 and qc == 0),
                                stop=(t == send and qc == 3),
                                skip_group_check=True,
                            )

            def emit_drain(pvs, heads, qb, sbi):
                for h, pv in zip(heads, pvs):
                    src = pv[:].rearrange("p (a x) -> p a x", x=128)[:, :, 0:65]
                    dst = O[:, qb * 4 : qb * 4 + 4, h : h + 1, :]
                    if sbi == 0:
                        nc.vector.tensor_copy(dst, src)
                    else:
                        nc.vector.tensor_add(dst, src, dst)
                if sbi == len(sbs) - 1 and heads[0] == H - 2:
                    # all of qb's O rows are final: queue epilogue units
                    for qt in range(qb * 4, qb * 4 + 4):
                        epi.append(mk_norm(qt))
                        epi.append(mk_tr(qt))
                        epi.append(mk_op(qt))

            # due-slot emission: PV lags its exp by 1 group-slot, drain-adds
            # lag by 2; drains are emitted before PVs within a slot so a new
            # iteration's chains (which may rotate onto a drained bank) are
            # emitted after the drain that reads it
            pend_pv = deque()  # (due_slot, args)
            pend_drain = deque()  # (due_slot, args)
            slot = 0

            def flush(cur):
                while pend_drain and pend_drain[0][0] <= cur:
                    emit_drain(*pend_drain.popleft()[1])
                while pend_pv and pend_pv[0][0] <= cur:
                    emit_pv(*pend_pv.popleft()[1])

            for sbi, s0, sn, qb, hp, groups in sched:
                hA, hB = 2 * hp, 2 * hp + 1
                pvA = ps_pv.tile([128, 512], F32, tag="pv", name="pvA")
                pvB = ps_pv.tile([128, 512], F32, tag="pv", name="pvB")
                for t0, tn in groups:
                    # deadline-paced projection units: stay ~one super-block
                    # ahead of attention; otherwise drain epilogue units
                    # HARD requirement: this group's kT/v2 tiles must have
                    # been emitted before the consuming matmuls (Tile deps
                    # only see already-emitted writers)
                    while units and coverage[0] < t0 + tn:
                        pop_unit()
                    # soft pacing: stay ~one super-block ahead, else epilogue
                    pops = 0
                    while (
                        units
                        and pops < 2
                        and coverage[0] < min(KT, s0 + sn + 2)
                    ):
                        pop_unit()
                        pops += 1
                    if pops == 0 and epi:
                        epi.popleft()()
                    # scores per head into separate 2-bank tiles so exp(A)
                    # overlaps the S matmuls of head B
                    pts = []
                    for hi, scp in ((0, ps_scA), (1, ps_scB)):
                        sc = scp.tile([128, 2, 512], F32, tag="sc", name="sc")
                        pt = ptp.tile([128, 2, 512], BF16, tag="pt", name="pt")
                        pts.append(pt)
                        for j in range(tn):
                            t = t0 + j
                            co = t * 128
                            nc.tensor.matmul(
                                sc[:, j, :],
                                kT[hi * 64 : hi * 64 + 64, hp, co : co + 128],
                                qT[hi * 64 : hi * 64 + 64, hp, qb * QB : (qb + 1) * QB],
                                start=True,
                                stop=True,
                            )
                        nc.scalar.activation(
                            pt[:, 0:tn, :], sc[:, 0:tn, :], AF.Exp
                        )
                    flush(slot)
                    pend_pv.append(
                        (slot + 1, (t0, tn, pts, (pvA, pvB), (hA, hB), s0, s0 + sn - 1))
                    )
                    slot += 1
                pend_drain.append((slot + 1, ((pvA, pvB), (hA, hB), qb, sbi)))

            # final flush in due order (a drain must follow its own PV)
            items = [(d, 0, a) for d, a in pend_drain] + [
                (d, 1, a) for d, a in pend_pv
            ]
            for d, ty, a in sorted(items, key=lambda x: (x[0], x[1])):
                (emit_drain if ty == 0 else emit_pv)(*a)
            pend_drain.clear()
            pend_pv.clear()
            while units:
                pop_unit()
            while epi:
                epi.popleft()()

    nc.compile()
    return nc


def kernel(x, context_tensor, mask, Wq, Wk, Wv, Wo, bo):
    import ml_dtypes
    from concourse.bass_utils import run_bass_kernel_spmd

    bf16 = ml_dtypes.bfloat16
    x = np.ascontiguousarray(np.asarray(x, dtype=np.float32).astype(bf16))
    context_tensor = np.asarray(context_tensor, dtype=np.float32).astype(bf16)
    mask = np.asarray(mask)
    Wq = np.ascontiguousarray(np.asarray(Wq, dtype=np.float32).astype(bf16))
    Wk = np.ascontiguousarray(np.asarray(Wk, dtype=np.float32).astype(bf16))
    Wv = np.ascontiguousarray(np.asarray(Wv, dtype=np.float32).astype(bf16))
    Wo = np.ascontiguousarray(np.asarray(Wo, dtype=np.float32).astype(bf16))
    bo = np.ascontiguousarray(np.asarray(bo, dtype=np.float32))

    # host-side context compaction using the mask
    meffs = [int(mask[b].sum()) for b in range(B)]
    m_pad = max(M_PAD_MIN, ((max(meffs) + 127) // 128) * 128)
    ctx_c = np.zeros((B, m_pad, CONTEXT_DIM), dtype=bf16)
    val = np.zeros((B, m_pad), dtype=np.float32)
    for b in range(B):
        idx = np.flatnonzero(mask[b])
        ctx_c[b, : len(idx)] = context_tensor[b, idx]
        val[b, : len(idx)] = 1.0

    if m_pad not in _compiled:
        _compiled[m_pad] = _build(m_pad)
    nc = _compiled[m_pad]

    rows_per_core = N // (NCORES // B)  # 1024
    in_maps = []
    for d in range(NCORES):
        b = d // (NCORES // B)
        r0 = (d % (NCORES // B)) * rows_per_core
        in_maps.append(
            {
                "xs": x[b, r0 : r0 + rows_per_core],
                "ctx": ctx_c[b],
                "valid": val[b],
                "Wq": Wq,
                "Wk": Wk,
                "Wv": Wv,
                "Wo": Wo,
                "bo": bo,
            }
        )

    res = run_bass_kernel_spmd(nc, in_maps, list(range(NCORES)))
    out = np.empty((B, N, QUERY_DIM), dtype=np.float32)
    for d in range(NCORES):
        b = d // (NCORES // B)
        r0 = (d % (NCORES // B)) * rows_per_core
        out[b, r0 : r0 + rows_per_core] = res.results[d]["out"]
    return out


# revision 34
# speedup vs baseline: 1.0055x; 1.0055x over previous
"""Cross-attention Bass kernel for 8 trn2 NeuronCores.

Sharding: core d handles batch b = d//4 and query rows [(d%4)*1024, (d%4+1)*1024)
of that batch, computing all 8 heads (no collectives). The context is compacted
on the host using the mask (masked rows dropped, zero-padded to a multiple of
128), which preserves softmax semantics exactly while nearly halving the
attention work.

Device dataflow (k-blocked, engine-balanced, software-pipelined):
  - Q^T = Wq^T x^T (bf16, softmax scale folded into the drain), K^T = Wk^T ctx^T
    (bf16), V natural = ctx^T-contracted with Wv (bf16) with a per-head "ones"
    column carrying the valid mask.
  - Scores per 2-ktile group, split by head into two 2-bank PSUM tiles so the
    exp of head A overlaps the score matmuls of head B (effective double
    buffering inside the 8-bank budget); one exp per head-group on ScalarE
    -> P^T bf16.
  - PV in NATURAL orientation: O[q, 65] += P^T-chunk.T @ [V|valid] with bf16
    operands (free dim 65 -> half the PE rows of the O^T orientation). Each
    PSUM bank holds 4 q-chunk chains as ONE accumulation group (start on the
    bank's first matmul, stop on its last). Chains span a 2-4 ktile
    super-block, then drain-add into an SBUF O accumulator on VectorE. The
    softmax denominator rides along as column 64, so normalization is a
    per-partition scalar multiply - no DRAM broadcast round-trips.
  - PV and drain-adds are emitted one group late (software pipeline), hiding
    the exp latency; K/V/ctx^T projection work is split into small units and
    deadline-paced into the attention group loop so the Tensor engine stays
    busy while ScalarE exps.
  - Epilogue: normalize on VectorE+ScalarE into packed tiles, PE-transpose,
    output projection, bias on VectorE, store via Pool-engine DMA.
"""
import numpy as np

B, N, M = 2, 4096, 4096
QUERY_DIM, CONTEXT_DIM = 512, 768
H, D = 8, 64
INNER = H * D  # 512
NCORES = 8
N_DEV = (B * N) // NCORES  # 1024 query rows per core
M_PAD_MIN = 256

_compiled = {}


def _build(m_pad):
    from collections import deque

    from concourse import bacc
    import concourse.bass as bass
    import concourse.mybir as mybir
    import concourse.tile as tile
    from concourse.masks import make_identity

    F32 = mybir.dt.float32
    F32R = mybir.dt.float32r
    BF16 = mybir.dt.bfloat16
    AF = mybir.ActivationFunctionType

    KT = m_pad // 128
    SCALE = float(D) ** -0.5
    CQ = QUERY_DIM // 128  # 4
    CC = CONTEXT_DIM // 128  # 6
    CI = INNER // 128  # 4
    NQT = N_DEV // 128  # 8
    QB = 512
    NQB = N_DEV // QB  # 2

    # super-blocks (PV accumulation chain extents): small first block so
    # attention starts early, then 4-ktile blocks
    sbs = []
    s = 0
    while s < KT:
        n = min(2 if s == 0 else 4, KT - s)
        sbs.append((s, n))
        s += n
    # projection j-blocks: pairs of ktiles; fold an odd trailing ktile into a
    # final 3-wide block so every K-projection matmul keeps free dim >= 256
    if KT % 2 == 1 and KT >= 3:
        jbs = [(b, 2) for b in range(0, KT - 3, 2)] + [(KT - 3, 3)]
    else:
        jbs = [(b, min(2, KT - b)) for b in range(0, KT, 2)]
    JW = 384 if (KT % 2 == 1 and KT >= 3) else 256  # widest j-block

    nc = bacc.Bacc()
    xs_d = nc.declare_dram_parameter("xs", [N_DEV, QUERY_DIM], BF16, isOutput=False)
    ctx_d = nc.declare_dram_parameter("ctx", [m_pad, CONTEXT_DIM], BF16, isOutput=False)
    val_d = nc.declare_dram_parameter("valid", [m_pad], F32, isOutput=False)
    wq_d = nc.declare_dram_parameter("Wq", [QUERY_DIM, INNER], BF16, isOutput=False)
    wk_d = nc.declare_dram_parameter("Wk", [CONTEXT_DIM, INNER], BF16, isOutput=False)
    wv_d = nc.declare_dram_parameter("Wv", [CONTEXT_DIM, INNER], BF16, isOutput=False)
    wo_d = nc.declare_dram_parameter("Wo", [INNER, QUERY_DIM], BF16, isOutput=False)
    bo_d = nc.declare_dram_parameter("bo", [QUERY_DIM], F32, isOutput=False)
    out_d = nc.declare_dram_parameter("out", [N_DEV, QUERY_DIM], F32, isOutput=True)

    with tile.TileContext(nc) as tc:
        with (
            tc.tile_pool(name="big", bufs=1) as big,
            tc.tile_pool(name="strm", bufs=3) as strm,
            tc.tile_pool(name="ld", bufs=4) as ld,
            tc.tile_pool(name="ptp", bufs=8) as ptp,
            tc.tile_pool(name="outp", bufs=2) as outp,
            tc.tile_pool(name="nrm", bufs=4) as nrm,
            tc.tile_pool(name="ps_scA", bufs=1, space="PSUM") as ps_scA,
            tc.tile_pool(name="ps_scB", bufs=1, space="PSUM") as ps_scB,
            tc.tile_pool(name="ps_pv", bufs=2, space="PSUM") as ps_pv,
            tc.tile_pool(name="ps_pj", bufs=2, space="PSUM") as ps_pj,
        ):
            # ---- persistent SBUF tiles ----
            qT = big.tile([128, CI, N_DEV], BF16, tag="qT", name="qT")
            kT = big.tile([128, CI, m_pad], BF16, tag="kT", name="kT")
            v2 = [
                big.tile([128, H * 65], BF16, tag=f"v2_{t}", name=f"v2_{t}")
                for t in range(KT)
            ]
            # O accumulator: [q-tile, head, 64 dims + denominator]
            O = big.tile([128, NQT, H, 65], F32R, tag="O", name="O")
            rcp = big.tile([128, NQT, H], F32, tag="rcp", name="rcp")
            wo = big.tile([128, CI, QUERY_DIM], BF16, tag="wo", name="wo")
            bo_bc = big.tile([128, QUERY_DIM], F32, tag="bo", name="bo")
            valid = big.tile([128, KT], F32, tag="valid", name="valid")
            oT = [
                big.tile([128, CI, QB], BF16, tag=f"oT{qb}", name=f"oT{qb}")
                for qb in range(NQB)
            ]
            wk = big.tile([128, CC, INNER], BF16, tag="wk", name="wk")
            wv = big.tile([128, CC, INNER], BF16, tag="wv", name="wv")
            identf = big.tile([128, 128], F32, tag="identf", name="identf")
            identb = big.tile([128, 128], BF16, tag="identb", name="identb")

            # ---- prologue: x^T and Q^T ----
            xs = big.tile([128, NQT, QUERY_DIM], BF16, tag="xs", name="xs")
            xT = big.tile([128, CQ, N_DEV], BF16, tag="xT", name="xT")
            wq = big.tile([128, CQ, INNER], BF16, tag="wq", name="wq")
            if True:
                xs_r = xs_d[:].rearrange("(t p) f -> p t f", p=128)
                nc.sync.dma_start(out=xs[:, 0:4, :], in_=xs_r[:, 0:4, :])
                nc.sync.dma_start(
                    out=wq[:], in_=wq_d[:].rearrange("(o p) f -> p o f", p=128)
                )
                nc.sync.dma_start(
                    out=wk[:], in_=wk_d[:].rearrange("(o p) f -> p o f", p=128)
                )
                nc.sync.dma_start(out=xs[:, 4:8, :], in_=xs_r[:, 4:8, :])
                nc.sync.dma_start(
                    out=wv[:], in_=wv_d[:].rearrange("(o p) f -> p o f", p=128)
                )
                nc.sync.dma_start(
                    out=valid[:], in_=val_d[:].rearrange("(t p) -> p t", p=128)
                )
                nc.sync.dma_start(
                    out=bo_bc[:],
                    in_=bass.AP(tensor=bo_d, offset=0, ap=[[0, 128], [1, QUERY_DIM]]),
                )
                make_identity(nc, identf[:])
                nc.vector.tensor_copy(identb[:], identf[:])

                # x^T then Q^T for the first q-half only; the second half is
                # deferred into filler units (qb1 consumes it several slots in)
                for qf in range(1):
                    for nt in range(qf * 4, qf * 4 + 4):
                        if nt % 2 == 0:
                            dst = ps_pj.tile([128, 512], BF16, tag="pj", name="pjx")
                        else:
                            dst = ps_pv.tile([128, 512], BF16, tag="pv", name="pvx")
                        for c in range(CQ):
                            nc.tensor.transpose(
                                dst[:, c * 128 : (c + 1) * 128],
                                xs[:, nt, c * 128 : (c + 1) * 128],
                                identb[:],
                            )
                        nc.vector.tensor_copy(
                            xT[:, :, nt * 128 : (nt + 1) * 128],
                            dst[:, 0 : CQ * 128].rearrange("p (c n) -> p c n", n=128),
                        )
                    for dc in range(CI):
                        psq = ps_pv.tile([128, 512], F32, tag="pv", name="psq")
                        for c in range(CQ):
                            nc.tensor.matmul(
                                psq[:],
                                wq[:, c, dc * 128 : (dc + 1) * 128],
                                xT[:, c, qf * QB : (qf + 1) * QB],
                                start=(c == 0),
                                stop=(c == CQ - 1),
                            )
                        nc.scalar.activation(
                            qT[:, dc, qf * QB : (qf + 1) * QB],
                            psq[:],
                            AF.Copy,
                            scale=SCALE,
                        )

            # ---- projection units (ctx^T, K^T, V), deadline-paced below ----
            ctxT_tiles = {}

            def mk_T(jbi, k):
                def t_unit():
                    b, nkt = jbs[jbi]
                    if k == 0:
                        ctxT_tiles[jbi] = strm.tile(
                            [128, CC, JW], BF16, tag="ctxT", name="ctxT"
                        )
                    ct = ctxT_tiles[jbi]
                    t = b + k
                    raw = ld.tile([128, CONTEXT_DIM], BF16, tag="ld", name="ld")
                    nc.sync.dma_start(
                        out=raw[:], in_=ctx_d[t * 128 : (t + 1) * 128, :]
                    )
                    for c0, cn in ((0, 4), (4, 2)):
                        pj = ps_pj.tile([128, 512], BF16, tag="pj", name="pjt")
                        for cl in range(cn):
                            nc.tensor.transpose(
                                pj[:, cl * 128 : (cl + 1) * 128],
                                raw[:, (c0 + cl) * 128 : (c0 + cl + 1) * 128],
                                identb[:],
                            )
                        nc.vector.tensor_copy(
                            ct[:, c0 : c0 + cn, k * 128 : (k + 1) * 128],
                            pj[:, 0 : cn * 128].rearrange("p (c n) -> p c n", n=128),
                        )

                return t_unit

            def mk_K(jbi, dc):
                # K^T for j-block jbi, one INNER chunk dc
                def k_unit():
                    b, nkt = jbs[jbi]
                    w = nkt * 128
                    ct = ctxT_tiles[jbi]
                    pj = ps_pj.tile([128, 512], F32, tag="pj", name="pjk")
                    for c in range(CC):
                        nc.tensor.matmul(
                            pj[:, 0:w],
                            wk[:, c, dc * 128 : (dc + 1) * 128],
                            ct[:, c, 0:w],
                            start=(c == 0),
                            stop=(c == CC - 1),
                        )
                    nc.vector.tensor_copy(
                        kT[:, dc, b * 128 : b * 128 + w], pj[:, 0:w]
                    )

                return k_unit

            def mk_V(jbi, k):
                def v_unit():
                    b, nkt = jbs[jbi]
                    ct = ctxT_tiles[jbi]
                    t = b + k
                    pj = ps_pj.tile([128, 512], F32, tag="pj", name="pjv")
                    sl = pj[:, 0:512]
                    for c in range(CC):
                        nc.tensor.matmul(
                            sl,
                            ct[:, c, k * 128 : (k + 1) * 128],
                            wv[:, c, :],
                            start=(c == 0),
                            stop=(c == CC - 1),
                        )
                    v2h = v2[t][:].rearrange("p (h c) -> p h c", c=65)
                    nc.vector.tensor_copy(
                        v2h[:, :, 0:64], sl.rearrange("p (h d) -> p h d", d=64)
                    )
                    nc.gpsimd.tensor_copy(
                        v2h[:, :, 64:65], valid[:, t : t + 1].to_broadcast([128, H, 1])
                    )

                return v_unit

            # (unit_fn, kt_covered_after) - coverage advances when the last
            # unit of a j-block has been emitted
            units = deque()
            for jbi, (b, nkt) in enumerate(jbs):
                seq = (
                    [mk_T(jbi, k) for k in range(nkt)]
                    + [mk_K(jbi, dc) for dc in range(CI)]
                    + [mk_V(jbi, k) for k in range(nkt)]
                )
                for i, fn in enumerate(seq):
                    units.append((fn, b + nkt if i == len(seq) - 1 else 0))

            def mk_xq(step):
                def xq_unit():
                    if step == 0:
                        for nt in range(4, 8):
                            dst = (ps_pj if nt % 2 == 0 else ps_pv).tile(
                                [128, 512], BF16,
                                tag="pj" if nt % 2 == 0 else "pv",
                                name="pvx2",
                            )
                            for c in range(CQ):
                                nc.tensor.transpose(
                                    dst[:, c * 128 : (c + 1) * 128],
                                    xs[:, nt, c * 128 : (c + 1) * 128],
                                    identb[:],
                                )
                            nc.vector.tensor_copy(
                                xT[:, :, nt * 128 : (nt + 1) * 128],
                                dst[:, 0 : CQ * 128].rearrange(
                                    "p (c n) -> p c n", n=128
                                ),
                            )
                    else:
                        dc = step - 1
                        psq = ps_pv.tile([128, 512], F32, tag="pv", name="psq2")
                        for c in range(CQ):
                            nc.tensor.matmul(
                                psq[:],
                                wq[:, c, dc * 128 : (dc + 1) * 128],
                                xT[:, c, QB : 2 * QB],
                                start=(c == 0),
                                stop=(c == CQ - 1),
                            )
                        nc.scalar.activation(
                            qT[:, dc, QB : 2 * QB], psq[:], AF.Copy, scale=SCALE
                        )

                return xq_unit

            for step in range(4, -1, -1):
                units.appendleft((mk_xq(step), 0))

            coverage = [0]

            def pop_unit():
                fn, cov = units.popleft()
                fn()
                if cov:
                    coverage[0] = cov

            # prologue: emit units covering super-block 0
            while units and coverage[0] < sbs[0][1]:
                pop_unit()

            nc.sync.dma_start(
                out=wo[:], in_=wo_d[:].rearrange("(o p) f -> p o f", p=128)
            )

            # ---- epilogue units ----
            onrm = {}
            epi = deque()

            def mk_norm(qt):
                def n_unit():
                    nc.vector.reciprocal(
                        rcp[:, qt : qt + 1, :], O[:, qt : qt + 1, :, 64:65]
                    )
                    st = nrm.tile([128, INNER], BF16, tag="onrm", name="onrm")
                    onrm[qt] = st
                    for h in range(H):
                        nc.vector.tensor_scalar_mul(
                            st[:, h * 64 : (h + 1) * 64],
                            O[:, qt : qt + 1, h : h + 1, 0:64],
                            rcp[:, qt : qt + 1, h : h + 1],
                        )

                return n_unit

            def mk_tr(qt):
                def tr_unit():
                    qb, qtl = qt // (NQT // NQB), qt % (NQT // NQB)
                    pj = ps_pj.tile([128, 512], BF16, tag="pj", name="pjtr")
                    for c in range(CI):
                        nc.tensor.transpose(
                            pj[:, c * 128 : (c + 1) * 128],
                            onrm[qt][:, c * 128 : (c + 1) * 128],
                            identb[:],
                        )
                    nc.vector.tensor_copy(
                        oT[qb][:, :, qtl * 128 : (qtl + 1) * 128],
                        pj[:, 0 : CI * 128].rearrange("p (c n) -> p c n", n=128),
                    )

                return tr_unit

            def mk_op(qt):
                def op_unit():
                    qb, qtl = qt // (NQT // NQB), qt % (NQT // NQB)
                    pj = ps_pj.tile([128, 512], F32, tag="pj", name="pjop")
                    for c in range(CI):
                        nc.tensor.matmul(
                            pj[:, 0:512],
                            oT[qb][:, c, qtl * 128 : (qtl + 1) * 128],
                            wo[:, c, :],
                            start=(c == 0),
                            stop=(c == CI - 1),
                        )
                    ot = outp.tile([128, QUERY_DIM], F32, tag="ot", name="ot")
                    nc.vector.tensor_add(ot[:], pj[:, 0:512], bo_bc[:])
                    nc.gpsimd.dma_start(
                        out=out_d[qt * 128 : (qt + 1) * 128, :], in_=ot[:]
                    )

                return op_unit

            # ---- main attention loop (software-pipelined across groups,
            # iterations, and super-blocks) ----
            sched = []
            for sbi, (s0, sn) in enumerate(sbs):
                groups = [(t0, min(2, s0 + sn - t0)) for t0 in range(s0, s0 + sn, 2)]
                for qb in range(NQB):
                    for hp in range(H // 2):
                        sched.append((sbi, s0, sn, qb, hp, groups))

            def emit_pv(t0, tn, pts, pvs, heads, s0, send):
                for hi, (h, pv) in enumerate(zip(heads, pvs)):
                    for j in range(tn):
                        t = t0 + j
                        for qc in range(4):
                            # one accumulation group per PSUM bank: start/stop
                            # only on the bank's first/last matmul of the sb
                            nc.tensor.matmul(
                                pv[:, qc * 128 : qc * 128 + 65],
                                pts[hi][:, j, qc * 128 : (qc + 1) * 128],
                                v2[t][:, h * 65 : h * 65 + 65],
                                start=(t == s0 and qc == 0),
                                stop=(t == send and qc == 3),
                                skip_group_check=True,
                            )

            def emit_drain(pvs, heads, qb, sbi):
                for h, pv in zip(heads, pvs):
                    src = pv[:].rearrange("p (a x) -> p a x", x=128)[:, :, 0:65]
                    dst = O[:, qb * 4 : qb * 4 + 4, h : h + 1, :]
                    if sbi == 0:
                        nc.vector.tensor_copy(dst, src)
                    else:
                        nc.vector.tensor_add(dst, src, dst)
                if sbi == len(sbs) - 1 and heads[0] == H - 2:
                    # all of qb's O rows are final: queue epilogue units
                    for qt in range(qb * 4, qb * 4 + 4):
                        epi.append(mk_norm(qt))
                        epi.append(mk_tr(qt))
                        epi.append(mk_op(qt))

            # due-slot emission: PV lags its exp by 1 group-slot, drain-adds
            # lag by 2; drains are emitted before PVs within a slot so a new
            # iteration's chains (which may rotate onto a drained bank) are
            # emitted after the drain that reads it
            pend_pv = deque()  # (due_slot, args)
            pend_drain = deque()  # (due_slot, args)
            slot = 0

            def flush(cur):
                while pend_drain and pend_drain[0][0] <= cur:
                    emit_drain(*pend_drain.popleft()[1])
                while pend_pv and pend_pv[0][0] <= cur:
                    emit_pv(*pend_pv.popleft()[1])

            for sbi, s0, sn, qb, hp, groups in sched:
                hA, hB = 2 * hp, 2 * hp + 1
                pvA = ps_pv.tile([128, 512], F32, tag="pv", name="pvA")
                pvB = ps_pv.tile([128, 512], F32, tag="pv", name="pvB")
                for t0, tn in groups:
                    # deadline-paced projection units: stay ~one super-block
                    # ahead of attention; otherwise drain epilogue units
                    # HARD requirement: this group's kT/v2 tiles must have
                    # been emitted before the consuming matmuls (Tile deps
                    # only see already-emitted writers)
                    while units and coverage[0] < t0 + tn:
                        pop_unit()
                    # soft pacing: stay ~one super-block ahead, else epilogue
                    pops = 0
                    while (
                        units
                        and pops < 1
                        and coverage[0] < min(KT, s0 + sn + 2)
                    ):
                        pop_unit()
                        pops += 1
                    if epi and (pops == 0 or not units):
                        epi.popleft()()
                    # scores per head into separate 2-bank tiles so exp(A)
                    # overlaps the S matmuls of head B
                    pts = []
                    for hi, scp in ((0, ps_scA), (1, ps_scB)):
                        sc = scp.tile([128, 2, 512], F32, tag="sc", name="sc")
                        pt = ptp.tile([128, 2, 512], BF16, tag="pt", name="pt")
                        pts.append(pt)
                        for j in range(tn):
                            t = t0 + j
                            co = t * 128
                            nc.tensor.matmul(
                                sc[:, j, :],
                                kT[hi * 64 : hi * 64 + 64, hp, co : co + 128],
                                qT[hi * 64 : hi * 64 + 64, hp, qb * QB : (qb + 1) * QB],
                                start=True,
                                stop=True,
                            )
                        nc.scalar.activation(
                            pt[:, 0:tn, :], sc[:, 0:tn, :], AF.Exp
                        )
                    flush(slot)
                    pend_pv.append(
                        (slot + 1, (t0, tn, pts, (pvA, pvB), (hA, hB), s0, s0 + sn - 1))
                    )
                    slot += 1
                pend_drain.append((slot + 1, ((pvA, pvB), (hA, hB), qb, sbi)))

            # final flush in due order (a drain must follow its own PV)
            items = [(d, 0, a) for d, a in pend_drain] + [
                (d, 1, a) for d, a in pend_pv
            ]
            for d, ty, a in sorted(items, key=lambda x: (x[0], x[1])):
                (emit_drain if ty == 0 else emit_pv)(*a)
            pend_drain.clear()
            pend_pv.clear()
            while units:
                pop_unit()
            while epi:
                epi.popleft()()

    nc.compile()
    return nc


def kernel(x, context_tensor, mask, Wq, Wk, Wv, Wo, bo):
    import ml_dtypes
    from concourse.bass_utils import run_bass_kernel_spmd

    bf16 = ml_dtypes.bfloat16
    x = np.ascontiguousarray(np.asarray(x, dtype=np.float32).astype(bf16))
    context_tensor = np.asarray(context_tensor, dtype=np.float32).astype(bf16)
    mask = np.asarray(mask)
    Wq = np.ascontiguousarray(np.asarray(Wq, dtype=np.float32).astype(bf16))
    Wk = np.ascontiguousarray(np.asarray(Wk, dtype=np.float32).astype(bf16))
    Wv = np.ascontiguousarray(np.asarray(Wv, dtype=np.float32).astype(bf16))
    Wo = np.ascontiguousarray(np.asarray(Wo, dtype=np.float32).astype(bf16))
    bo = np.ascontiguousarray(np.asarray(bo, dtype=np.float32))

    # host-side context compaction using the mask
    meffs = [int(mask[b].sum()) for b in range(B)]
    m_pad = max(M_PAD_MIN, ((max(meffs) + 127) // 128) * 128)
    ctx_c = np.zeros((B, m_pad, CONTEXT_DIM), dtype=bf16)
    val = np.zeros((B, m_pad), dtype=np.float32)
    for b in range(B):
        idx = np.flatnonzero(mask[b])
        ctx_c[b, : len(idx)] = context_tensor[b, idx]
        val[b, : len(idx)] = 1.0

    if m_pad not in _compiled:
        _compiled[m_pad] = _build(m_pad)
    nc = _compiled[m_pad]

    rows_per_core = N // (NCORES // B)  # 1024
    in_maps = []
    for d in range(NCORES):
        b = d // (NCORES // B)
        r0 = (d % (NCORES // B)) * rows_per_core
        in_maps.append(
            {
                "xs": x[b, r0 : r0 + rows_per_core],
                "ctx": ctx_c[b],
                "valid": val[b],
                "Wq": Wq,
                "Wk": Wk,
                "Wv": Wv,
                "Wo": Wo,
                "bo": bo,
            }
        )

    res = run_bass_kernel_spmd(nc, in_maps, list(range(NCORES)))
    out = np.empty((B, N, QUERY_DIM), dtype=np.float32)
    for d in range(NCORES):
        b = d // (NCORES // B)
        r0 = (d % (NCORES // B)) * rows_per_core
        out[b, r0 : r0 + rows_per_core] = res.results[d]["out"]
    return out


# revision 35
# speedup vs baseline: 1.0068x; 1.0013x over previous
"""Cross-attention Bass kernel for 8 trn2 NeuronCores.

Sharding: core d handles batch b = d//4 and query rows [(d%4)*1024, (d%4+1)*1024)
of that batch, computing all 8 heads (no collectives). The context is compacted
on the host using the mask (masked rows dropped, zero-padded to a multiple of
128), which preserves softmax semantics exactly while nearly halving the
attention work.

Device dataflow (k-blocked, engine-balanced, software-pipelined):
  - Q^T = Wq^T x^T (bf16, softmax scale folded into the drain), K^T = Wk^T ctx^T
    (bf16), V natural = ctx^T-contracted with Wv (bf16) with a per-head "ones"
    column carrying the valid mask.
  - Scores per 2-ktile group, split by head into two 2-bank PSUM tiles so the
    exp of head A overlaps the score matmuls of head B (effective double
    buffering inside the 8-bank budget); one exp per head-group on ScalarE
    -> P^T bf16.
  - PV in NATURAL orientation: O[q, 65] += P^T-chunk.T @ [V|valid] with bf16
    operands (free dim 65 -> half the PE rows of the O^T orientation). Each
    PSUM bank holds 4 q-chunk chains as ONE accumulation group (start on the
    bank's first matmul, stop on its last). Chains span a 2-4 ktile
    super-block, then drain-add into an SBUF O accumulator on VectorE. The
    softmax denominator rides along as column 64, so normalization is a
    per-partition scalar multiply - no DRAM broadcast round-trips.
  - PV and drain-adds are emitted one group late (software pipeline), hiding
    the exp latency; K/V/ctx^T projection work is split into small units and
    deadline-paced into the attention group loop so the Tensor engine stays
    busy while ScalarE exps.
  - Epilogue: normalize on VectorE+ScalarE into packed tiles, PE-transpose,
    output projection, bias on VectorE, store via Pool-engine DMA.
"""
import numpy as np

B, N, M = 2, 4096, 4096
QUERY_DIM, CONTEXT_DIM = 512, 768
H, D = 8, 64
INNER = H * D  # 512
NCORES = 8
N_DEV = (B * N) // NCORES  # 1024 query rows per core
M_PAD_MIN = 256

_compiled = {}


def _build(m_pad):
    from collections import deque

    from concourse import bacc
    import concourse.bass as bass
    import concourse.mybir as mybir
    import concourse.tile as tile
    from concourse.masks import make_identity

    F32 = mybir.dt.float32
    F32R = mybir.dt.float32r
    BF16 = mybir.dt.bfloat16
    AF = mybir.ActivationFunctionType

    KT = m_pad // 128
    SCALE = float(D) ** -0.5
    CQ = QUERY_DIM // 128  # 4
    CC = CONTEXT_DIM // 128  # 6
    CI = INNER // 128  # 4
    NQT = N_DEV // 128  # 8
    QB = 512
    NQB = N_DEV // QB  # 2

    # super-blocks (PV accumulation chain extents): small first block so
    # attention starts early, then 4-ktile blocks
    sbs = []
    s = 0
    while s < KT:
        n = min(2 if s == 0 else 4, KT - s)
        sbs.append((s, n))
        s += n
    # projection j-blocks: pairs of ktiles; fold an odd trailing ktile into a
    # final 3-wide block so every K-projection matmul keeps free dim >= 256
    if KT % 2 == 1 and KT >= 3:
        jbs = [(b, 2) for b in range(0, KT - 3, 2)] + [(KT - 3, 3)]
    else:
        jbs = [(b, min(2, KT - b)) for b in range(0, KT, 2)]
    JW = 384 if (KT % 2 == 1 and KT >= 3) else 256  # widest j-block

    nc = bacc.Bacc()
    xs_d = nc.declare_dram_parameter("xs", [N_DEV, QUERY_DIM], BF16, isOutput=False)
    ctx_d = nc.declare_dram_parameter("ctx", [m_pad, CONTEXT_DIM], BF16, isOutput=False)
    val_d = nc.declare_dram_parameter("valid", [m_pad], F32, isOutput=False)
    wq_d = nc.declare_dram_parameter("Wq", [QUERY_DIM, INNER], BF16, isOutput=False)
    wk_d = nc.declare_dram_parameter("Wk", [CONTEXT_DIM, INNER], BF16, isOutput=False)
    wv_d = nc.declare_dram_parameter("Wv", [CONTEXT_DIM, INNER], BF16, isOutput=False)
    wo_d = nc.declare_dram_parameter("Wo", [INNER, QUERY_DIM], BF16, isOutput=False)
    bo_d = nc.declare_dram_parameter("bo", [QUERY_DIM], F32, isOutput=False)
    out_d = nc.declare_dram_parameter("out", [N_DEV, QUERY_DIM], F32, isOutput=True)

    with tile.TileContext(nc) as tc:
        with (
            tc.tile_pool(name="big", bufs=1) as big,
            tc.tile_pool(name="strm", bufs=3) as strm,
            tc.tile_pool(name="ld", bufs=4) as ld,
            tc.tile_pool(name="ptp", bufs=8) as ptp,
            tc.tile_pool(name="outp", bufs=2) as outp,
            tc.tile_pool(name="nrm", bufs=4) as nrm,
            tc.tile_pool(name="ps_scA", bufs=1, space="PSUM") as ps_scA,
            tc.tile_pool(name="ps_scB", bufs=1, space="PSUM") as ps_scB,
            tc.tile_pool(name="ps_pv", bufs=2, space="PSUM") as ps_pv,
            tc.tile_pool(name="ps_pj", bufs=2, space="PSUM") as ps_pj,
        ):
            # ---- persistent SBUF tiles ----
            qT = big.tile([128, CI, N_DEV], BF16, tag="qT", name="qT")
            kT = big.tile([128, CI, m_pad], BF16, tag="kT", name="kT")
            v2 = [
                big.tile([128, H * 65], BF16, tag=f"v2_{t}", name=f"v2_{t}")
                for t in range(KT)
            ]
            # O accumulator: [q-tile, head, 64 dims + denominator]
            O = big.tile([128, NQT, H, 65], F32R, tag="O", name="O")
            rcp = big.tile([128, NQT, H], F32, tag="rcp", name="rcp")
            wo = big.tile([128, CI, QUERY_DIM], BF16, tag="wo", name="wo")
            bo_bc = big.tile([128, QUERY_DIM], F32, tag="bo", name="bo")
            valid = big.tile([128, KT], F32, tag="valid", name="valid")
            oT = [
                big.tile([128, CI, QB], BF16, tag=f"oT{qb}", name=f"oT{qb}")
                for qb in range(NQB)
            ]
            wk = big.tile([128, CC, INNER], BF16, tag="wk", name="wk")
            wv = big.tile([128, CC, INNER], BF16, tag="wv", name="wv")
            identf = big.tile([128, 128], F32, tag="identf", name="identf")
            identb = big.tile([128, 128], BF16, tag="identb", name="identb")

            # ---- prologue: x^T and Q^T ----
            xs = big.tile([128, NQT, QUERY_DIM], BF16, tag="xs", name="xs")
            xT = big.tile([128, CQ, N_DEV], BF16, tag="xT", name="xT")
            wq = big.tile([128, CQ, INNER], BF16, tag="wq", name="wq")
            if True:
                xs_r = xs_d[:].rearrange("(t p) f -> p t f", p=128)
                nc.sync.dma_start(out=xs[:, 0:4, :], in_=xs_r[:, 0:4, :])
                nc.sync.dma_start(
                    out=wq[:], in_=wq_d[:].rearrange("(o p) f -> p o f", p=128)
                )
                nc.sync.dma_start(
                    out=wk[:], in_=wk_d[:].rearrange("(o p) f -> p o f", p=128)
                )
                nc.sync.dma_start(out=xs[:, 4:8, :], in_=xs_r[:, 4:8, :])
                nc.sync.dma_start(
                    out=wv[:], in_=wv_d[:].rearrange("(o p) f -> p o f", p=128)
                )
                nc.sync.dma_start(
                    out=valid[:], in_=val_d[:].rearrange("(t p) -> p t", p=128)
                )
                nc.sync.dma_start(
                    out=bo_bc[:],
                    in_=bass.AP(tensor=bo_d, offset=0, ap=[[0, 128], [1, QUERY_DIM]]),
                )
                make_identity(nc, identf[:])
                nc.vector.tensor_copy(identb[:], identf[:])

                # x^T then Q^T for the first q-half only; the second half is
                # deferred into filler units (qb1 consumes it several slots in)
                for qf in range(1):
                    for nt in range(qf * 4, qf * 4 + 4):
                        if nt % 2 == 0:
                            dst = ps_pj.tile([128, 512], BF16, tag="pj", name="pjx")
                        else:
                            dst = ps_pv.tile([128, 512], BF16, tag="pv", name="pvx")
                        for c in range(CQ):
                            nc.tensor.transpose(
                                dst[:, c * 128 : (c + 1) * 128],
                                xs[:, nt, c * 128 : (c + 1) * 128],
                                identb[:],
                            )
                        nc.vector.tensor_copy(
                            xT[:, :, nt * 128 : (nt + 1) * 128],
                            dst[:, 0 : CQ * 128].rearrange("p (c n) -> p c n", n=128),
                        )
                    for dc in range(CI):
                        psq = ps_pv.tile([128, 512], F32, tag="pv", name="psq")
                        for c in range(CQ):
                            nc.tensor.matmul(
                                psq[:],
                                wq[:, c, dc * 128 : (dc + 1) * 128],
                                xT[:, c, qf * QB : (qf + 1) * QB],
                                start=(c == 0),
                                stop=(c == CQ - 1),
                            )
                        nc.scalar.activation(
                            qT[:, dc, qf * QB : (qf + 1) * QB],
                            psq[:],
                            AF.Copy,
                            scale=SCALE,
                        )

            # ---- projection units (ctx^T, K^T, V), deadline-paced below ----
            ctxT_tiles = {}

            def mk_T(jbi, chalf):
                # ctx^T via XBAR DMA transpose straight from DRAM (bf16):
                # 3 column-chunks per unit, no PE/DVE work at all
                def t_unit():
                    b, nkt = jbs[jbi]
                    w = nkt * 128
                    if chalf == 0:
                        ctxT_tiles[jbi] = strm.tile(
                            [128, CC, JW], BF16, tag="ctxT", name="ctxT"
                        )
                    ct = ctxT_tiles[jbi]
                    for c in range(chalf * 3, chalf * 3 + 3):
                        nc.sync.dma_start(
                            out=ct[:, c : c + 1, 0:w],
                            in_=ctx_d[
                                b * 128 : b * 128 + w, c * 128 : (c + 1) * 128
                            ],
                            transpose=True,
                        )

                return t_unit

            def mk_K(jbi, dc):
                # K^T for j-block jbi, one INNER chunk dc
                def k_unit():
                    b, nkt = jbs[jbi]
                    w = nkt * 128
                    ct = ctxT_tiles[jbi]
                    pj = ps_pj.tile([128, 512], F32, tag="pj", name="pjk")
                    for c in range(CC):
                        nc.tensor.matmul(
                            pj[:, 0:w],
                            wk[:, c, dc * 128 : (dc + 1) * 128],
                            ct[:, c, 0:w],
                            start=(c == 0),
                            stop=(c == CC - 1),
                        )
                    nc.vector.tensor_copy(
                        kT[:, dc, b * 128 : b * 128 + w], pj[:, 0:w]
                    )

                return k_unit

            def mk_V(jbi, k):
                def v_unit():
                    b, nkt = jbs[jbi]
                    ct = ctxT_tiles[jbi]
                    t = b + k
                    pj = ps_pj.tile([128, 512], F32, tag="pj", name="pjv")
                    sl = pj[:, 0:512]
                    for c in range(CC):
                        nc.tensor.matmul(
                            sl,
                            ct[:, c, k * 128 : (k + 1) * 128],
                            wv[:, c, :],
                            start=(c == 0),
                            stop=(c == CC - 1),
                        )
                    v2h = v2[t][:].rearrange("p (h c) -> p h c", c=65)
                    nc.vector.tensor_copy(
                        v2h[:, :, 0:64], sl.rearrange("p (h d) -> p h d", d=64)
                    )
                    nc.gpsimd.tensor_copy(
                        v2h[:, :, 64:65], valid[:, t : t + 1].to_broadcast([128, H, 1])
                    )

                return v_unit

            # (unit_fn, kt_covered_after) - coverage advances when the last
            # unit of a j-block has been emitted
            units = deque()
            for jbi, (b, nkt) in enumerate(jbs):
                seq = (
                    [mk_T(jbi, 0), mk_T(jbi, 1)]
                    + [mk_K(jbi, dc) for dc in range(CI)]
                    + [mk_V(jbi, k) for k in range(nkt)]
                )
                for i, fn in enumerate(seq):
                    units.append((fn, b + nkt if i == len(seq) - 1 else 0))

            def mk_xq(step):
                def xq_unit():
                    if step == 0:
                        for nt in range(4, 8):
                            dst = (ps_pj if nt % 2 == 0 else ps_pv).tile(
                                [128, 512], BF16,
                                tag="pj" if nt % 2 == 0 else "pv",
                                name="pvx2",
                            )
                            for c in range(CQ):
                                nc.tensor.transpose(
                                    dst[:, c * 128 : (c + 1) * 128],
                                    xs[:, nt, c * 128 : (c + 1) * 128],
                                    identb[:],
                                )
                            nc.vector.tensor_copy(
                                xT[:, :, nt * 128 : (nt + 1) * 128],
                                dst[:, 0 : CQ * 128].rearrange(
                                    "p (c n) -> p c n", n=128
                                ),
                            )
                    else:
                        dc = step - 1
                        psq = ps_pv.tile([128, 512], F32, tag="pv", name="psq2")
                        for c in range(CQ):
                            nc.tensor.matmul(
                                psq[:],
                                wq[:, c, dc * 128 : (dc + 1) * 128],
                                xT[:, c, QB : 2 * QB],
                                start=(c == 0),
                                stop=(c == CQ - 1),
                            )
                        nc.scalar.activation(
                            qT[:, dc, QB : 2 * QB], psq[:], AF.Copy, scale=SCALE
                        )

                return xq_unit

            for step in range(4, -1, -1):
                units.appendleft((mk_xq(step), 0))

            coverage = [0]

            def pop_unit():
                fn, cov = units.popleft()
                fn()
                if cov:
                    coverage[0] = cov

            # prologue: emit units covering super-block 0
            while units and coverage[0] < sbs[0][1]:
                pop_unit()

            nc.sync.dma_start(
                out=wo[:], in_=wo_d[:].rearrange("(o p) f -> p o f", p=128)
            )

            # ---- epilogue units ----
            onrm = {}
            epi = deque()

            def mk_norm(qt):
                def n_unit():
                    nc.vector.reciprocal(
                        rcp[:, qt : qt + 1, :], O[:, qt : qt + 1, :, 64:65]
                    )
                    st = nrm.tile([128, INNER], BF16, tag="onrm", name="onrm")
                    onrm[qt] = st
                    for h in range(H):
                        nc.vector.tensor_scalar_mul(
                            st[:, h * 64 : (h + 1) * 64],
                            O[:, qt : qt + 1, h : h + 1, 0:64],
                            rcp[:, qt : qt + 1, h : h + 1],
                        )

                return n_unit

            def mk_tr(qt):
                def tr_unit():
                    qb, qtl = qt // (NQT // NQB), qt % (NQT // NQB)
                    pj = ps_pj.tile([128, 512], BF16, tag="pj", name="pjtr")
                    for c in range(CI):
                        nc.tensor.transpose(
                            pj[:, c * 128 : (c + 1) * 128],
                            onrm[qt][:, c * 128 : (c + 1) * 128],
                            identb[:],
                        )
                    nc.vector.tensor_copy(
                        oT[qb][:, :, qtl * 128 : (qtl + 1) * 128],
                        pj[:, 0 : CI * 128].rearrange("p (c n) -> p c n", n=128),
                    )

                return tr_unit

            def mk_op(qt):
                def op_unit():
                    qb, qtl = qt // (NQT // NQB), qt % (NQT // NQB)
                    pj = ps_pj.tile([128, 512], F32, tag="pj", name="pjop")
                    for c in range(CI):
                        nc.tensor.matmul(
                            pj[:, 0:512],
                            oT[qb][:, c, qtl * 128 : (qtl + 1) * 128],
                            wo[:, c, :],
                            start=(c == 0),
                            stop=(c == CI - 1),
                        )
                    ot = outp.tile([128, QUERY_DIM], F32, tag="ot", name="ot")
                    nc.vector.tensor_add(ot[:], pj[:, 0:512], bo_bc[:])
                    nc.gpsimd.dma_start(
                        out=out_d[qt * 128 : (qt + 1) * 128, :], in_=ot[:]
                    )

                return op_unit

            # ---- main attention loop (software-pipelined across groups,
            # iterations, and super-blocks) ----
            sched = []
            for sbi, (s0, sn) in enumerate(sbs):
                groups = [(t0, min(2, s0 + sn - t0)) for t0 in range(s0, s0 + sn, 2)]
                for qb in range(NQB):
                    for hp in range(H // 2):
                        sched.append((sbi, s0, sn, qb, hp, groups))

            def emit_pv(t0, tn, pts, pvs, heads, s0, send):
                for hi, (h, pv) in enumerate(zip(heads, pvs)):
                    for j in range(tn):
                        t = t0 + j
                        for qc in range(4):
                            # one accumulation group per PSUM bank: start/stop
                            # only on the bank's first/last matmul of the sb
                            nc.tensor.matmul(
                                pv[:, qc * 128 : qc * 128 + 65],
                                pts[hi][:, j, qc * 128 : (qc + 1) * 128],
                                v2[t][:, h * 65 : h * 65 + 65],
                                start=(t == s0 and qc == 0),
                                stop=(t == send and qc == 3),
                                skip_group_check=True,
                            )

            def emit_drain(pvs, heads, qb, sbi):
                for h, pv in zip(heads, pvs):
                    src = pv[:].rearrange("p (a x) -> p a x", x=128)[:, :, 0:65]
                    dst = O[:, qb * 4 : qb * 4 + 4, h : h + 1, :]
                    if sbi == 0:
                        nc.vector.tensor_copy(dst, src)
                    else:
                        nc.vector.tensor_add(dst, src, dst)
                if sbi == len(sbs) - 1 and heads[0] == H - 2:
                    # all of qb's O rows are final: queue epilogue units
                    for qt in range(qb * 4, qb * 4 + 4):
                        epi.append(mk_norm(qt))
                        epi.append(mk_tr(qt))
                        epi.append(mk_op(qt))

            # due-slot emission: PV lags its exp by 1 group-slot, drain-adds
            # lag by 2; drains are emitted before PVs within a slot so a new
            # iteration's chains (which may rotate onto a drained bank) are
            # emitted after the drain that reads it
            pend_pv = deque()  # (due_slot, args)
            pend_drain = deque()  # (due_slot, args)
            slot = 0

            def flush(cur):
                while pend_drain and pend_drain[0][0] <= cur:
                    emit_drain(*pend_drain.popleft()[1])
                while pend_pv and pend_pv[0][0] <= cur:
                    emit_pv(*pend_pv.popleft()[1])

            for sbi, s0, sn, qb, hp, groups in sched:
                hA, hB = 2 * hp, 2 * hp + 1
                pvA = ps_pv.tile([128, 512], F32, tag="pv", name="pvA")
                pvB = ps_pv.tile([128, 512], F32, tag="pv", name="pvB")
                for t0, tn in groups:
                    # deadline-paced projection units: stay ~one super-block
                    # ahead of attention; otherwise drain epilogue units
                    # HARD requirement: this group's kT/v2 tiles must have
                    # been emitted before the consuming matmuls (Tile deps
                    # only see already-emitted writers)
                    while units and coverage[0] < t0 + tn:
                        pop_unit()
                    # soft pacing: stay ~one super-block ahead, else epilogue
                    pops = 0
                    while (
                        units
                        and pops < 1
                        and coverage[0] < min(KT, s0 + sn + 2)
                    ):
                        pop_unit()
                        pops += 1
                    if epi and (pops == 0 or not units):
                        epi.popleft()()
                    # scores per head into separate 2-bank tiles so exp(A)
                    # overlaps the S matmuls of head B
                    pts = []
                    for hi, scp in ((0, ps_scA), (1, ps_scB)):
                        sc = scp.tile([128, 2, 512], F32, tag="sc", name="sc")
                        pt = ptp.tile([128, 2, 512], BF16, tag="pt", name="pt")
                        pts.append(pt)
                        for j in range(tn):
                            t = t0 + j
                            co = t * 128
                            nc.tensor.matmul(
                                sc[:, j, :],
                                kT[hi * 64 : hi * 64 + 64, hp, co : co + 128],
                                qT[hi * 64 : hi * 64 + 64, hp, qb * QB : (qb + 1) * QB],
                                start=True,
                                stop=True,
                            )
                        nc.scalar.activation(
                            pt[:, 0:tn, :], sc[:, 0:tn, :], AF.Exp
                        )
                    flush(slot)
                    pend_pv.append(
                        (slot + 1, (t0, tn, pts, (pvA, pvB), (hA, hB), s0, s0 + sn - 1))
                    )
                    slot += 1
                pend_drain.append((slot + 1, ((pvA, pvB), (hA, hB), qb, sbi)))

            # final flush in due order (a drain must follow its own PV)
            items = [(d, 0, a) for d, a in pend_drain] + [
                (d, 1, a) for d, a in pend_pv
            ]
            for d, ty, a in sorted(items, key=lambda x: (x[0], x[1])):
                (emit_drain if ty == 0 else emit_pv)(*a)
            pend_drain.clear()
            pend_pv.clear()
            while units:
                pop_unit()
            while epi:
                epi.popleft()()

    nc.compile()
    return nc


def kernel(x, context_tensor, mask, Wq, Wk, Wv, Wo, bo):
    import ml_dtypes
    from concourse.bass_utils import run_bass_kernel_spmd

    bf16 = ml_dtypes.bfloat16
    x = np.ascontiguousarray(np.asarray(x, dtype=np.float32).astype(bf16))
    context_tensor = np.asarray(context_tensor, dtype=np.float32).astype(bf16)
    mask = np.asarray(mask)
    Wq = np.ascontiguousarray(np.asarray(Wq, dtype=np.float32).astype(bf16))
    Wk = np.ascontiguousarray(np.asarray(Wk, dtype=np.float32).astype(bf16))
    Wv = np.ascontiguousarray(np.asarray(Wv, dtype=np.float32).astype(bf16))
    Wo = np.ascontiguousarray(np.asarray(Wo, dtype=np.float32).astype(bf16))
    bo = np.ascontiguousarray(np.asarray(bo, dtype=np.float32))

    # host-side context compaction using the mask
    meffs = [int(mask[b].sum()) for b in range(B)]
    m_pad = max(M_PAD_MIN, ((max(meffs) + 127) // 128) * 128)
    ctx_c = np.zeros((B, m_pad, CONTEXT_DIM), dtype=bf16)
    val = np.zeros((B, m_pad), dtype=np.float32)
    for b in range(B):
        idx = np.flatnonzero(mask[b])
        ctx_c[b, : len(idx)] = context_tensor[b, idx]
        val[b, : len(idx)] = 1.0

    if m_pad not in _compiled:
        _compiled[m_pad] = _build(m_pad)
    nc = _compiled[m_pad]

    rows_per_core = N // (NCORES // B)  # 1024
    in_maps = []
    for d in range(NCORES):
        b = d // (NCORES // B)
        r0 = (d % (NCORES // B)) * rows_per_core
        in_maps.append(
            {
                "xs": x[b, r0 : r0 + rows_per_core],
                "ctx": ctx_c[b],
                "valid": val[b],
                "Wq": Wq,
                "Wk": Wk,
                "Wv": Wv,
                "Wo": Wo,
                "bo": bo,
            }
        )

    res = run_bass_kernel_spmd(nc, in_maps, list(range(NCORES)))
    out = np.empty((B, N, QUERY_DIM), dtype=np.float32)
    for d in range(NCORES):
        b = d // (NCORES // B)
        r0 = (d % (NCORES // B)) * rows_per_core
        out[b, r0 : r0 + rows_per_core] = res.results[d]["out"]
    return out
